# revision 14
# baseline (speedup 1.0000x reference)
"""CountSketch kernel for Trainium2 (8 NeuronCores, SPMD data-parallel).

out[b, i_hash[j]] += x[b, j] * s_hash[j]
  x: [4096, 16384] f32, s_hash: [16384] f32, i_hash: [16384] int64 -> out [4096, 1024] f32

Strategy (batch-sharded, host-sorted bf16 layout, sequential DMA):
  - shard x by batch across 8 cores (512 rows each).
  - host computes (from the tiny i_hash/s_hash vectors) a bucket-sorted
    column order `perm`; x columns are permuted to that order, cast to
    bf16, and laid out host-side as [128 partitions, 65536] so the chunk
    for sorted position c*128+p, batch b sits at [p, c*512+b]: every
    device DMA tile is a contiguous per-partition-line slice (no gather).
  - banded +/-1 weight blocks (signs folded in) map each sorted 128-row
    chunk into its PSUM bank partitions; blocks are bf16 and only as wide
    as the PE column-tile constraints allow (base in {0,32,64}, width
    {32,64,128}).
  - each core accumulates out^T = [1024 f, 512 b] across the 128 chunks
    directly in PSUM (8 banks x [128, 512] = exactly all of PSUM); banks
    are closed, copied (cast to bf16) and DMA'd out as soon as the sorted
    stream passes their feature range, overlapping with later matmuls.
  - x tiles taper at the end (8,8,...,4,2,1,1 chunks) so the post-DMA
    matmul+drain tail is short.
  - host transposes/concatenates the 8 outT shards into [4096, 1024] f32.
"""
import numpy as np
import ml_dtypes
from contextlib import ExitStack

import concourse.bacc as bacc
import concourse.tile as tile
from concourse import mybir
from concourse import bass_utils

D_IN = 16384
D_F = 1024
B = 4096
NCORES = 8
BSH = B // NCORES          # 512 batch rows per core
CHUNK = 128                # sorted rows per matmul chunk
N_CHUNKS = D_IN // CHUNK   # 128
XCOLS = (D_IN // CHUNK) * BSH  # 65536 cols per partition of the x layout

# chunks per DMA tile: big steady-state tiles, tapered tail
SLOT_PLAN = [8] * 15 + [4, 2, 1, 1]
assert sum(SLOT_PLAN) == N_CHUNKS

F32 = mybir.dt.float32
BF16 = mybir.dt.bfloat16
FP8 = mybir.dt.float8e4   # weights dtype: signs +/-1 are exact in e4m3;
W_NP_DT = ml_dtypes.float8_e4m3  # HW-verified correct as lhsT vs bf16 rhs

MODE = "partial"           # narrow col-tiled weight blocks ("full" = [128,128])
OUT_BF16 = True            # write outT in bf16 (halves output DMA)
XBUFS = 7                  # in-flight x tiles (covers PE p-state ramp lag)


def _windows_for(fl_min, fl_max):
    """Minimal legal (p0, M) PE column windows covering [fl_min, fl_max].

    Legal combos: (0,32) (32,32) (64,32) (0,64) (64,64) (0,128).
    Returns disjoint windows covering the range.
    """
    singles = [(0, 32), (32, 32), (64, 32), (0, 64), (64, 64), (0, 128)]
    for p0, m in singles:
        if p0 <= fl_min and fl_max < p0 + m:
            return [(p0, m)]
    quads = sorted(set(range(fl_min // 32, fl_max // 32 + 1)))
    wins = []
    for q in quads:
        if q == 3:
            if (64, 32) in wins:
                wins.remove((64, 32))
            if (64, 64) not in wins:
                wins.append((64, 64))
        else:
            covered = any(p0 <= q * 32 and (q + 1) * 32 <= p0 + m for p0, m in wins)
            if not covered:
                wins.append((q * 32, 32))
    return wins


def _build_metadata(i_hash: np.ndarray, s_hash: np.ndarray):
    """Sort columns by bucket; build per-chunk banded weight blocks.

    Returns (perm, r_all, by_chunk, close_after): by_chunk[c] lists
    (bank, p0, M, off) matmul descriptors; r_all is the packed [128, total]
    bf16 weight matrix (col 0..127 = zero block); close_after[c] lists
    banks whose final touch is chunk c.
    """
    i_hash = np.asarray(i_hash).astype(np.int64).ravel()
    s_hash = np.asarray(s_hash).astype(np.float32).ravel()
    perm = np.argsort(i_hash, kind="stable")
    f_sorted = i_hash[perm]
    s_sorted = s_hash[perm]

    blocks = [np.zeros((CHUNK, CHUNK), np.float32)]  # zero block @ col 0
    off = CHUNK
    by_chunk = {}
    last_touch = {}       # bank -> last chunk touching it
    half_touch = {}       # (bank, half) -> last chunk touching that 64-range
    for c in range(N_CHUNKS):
        fs = f_sorted[c * CHUNK:(c + 1) * CHUNK]
        ss = s_sorted[c * CHUNK:(c + 1) * CHUNK]
        descs = []
        for h in np.unique(fs // CHUNK):
            sel = (fs // CHUNK) == h
            fl = (fs[sel] - h * CHUNK).astype(np.int64)  # local f in [0,128)
            rows = np.nonzero(sel)[0]
            last_touch[int(h)] = c
            for k in (0, 1):
                if np.any((fl >= 64 * k) & (fl < 64 * (k + 1))):
                    half_touch[(int(h), k)] = c
            if MODE == "full":
                wins = [(0, CHUNK)]
            else:
                wins = _windows_for(int(fl.min()), int(fl.max()))
            for (p0, m) in wins:
                wsel = (fl >= p0) & (fl < p0 + m)
                if not np.any(wsel):
                    continue
                R = np.zeros((CHUNK, m), np.float32)
                R[rows[wsel], fl[wsel] - p0] = ss[sel][wsel]
                blocks.append(R)
                descs.append((int(h), p0, m, off))
                off += m
        by_chunk[c] = descs
    r_all = np.concatenate(blocks, axis=1).astype(W_NP_DT)
    # Per-chunk drain schedule: close_half[c] lists (bank, half) 64-row
    # drains; stop_at[c] lists banks whose single 1-col stop matmul (sim-only
    # bookkeeping) is emitted at c — the bank's first drain point. Later
    # matmuls into the bank's other half accumulate fine (skip_group_check,
    # and stop is a no-op on hardware). Halves a bank never touches drain
    # zeros at the bank's last-touch chunk.
    close_half = {c: [] for c in range(N_CHUNKS)}
    stop_at = {c: [] for c in range(N_CHUNKS)}
    for h, c_last in last_touch.items():
        c0 = half_touch.get((h, 0), c_last)
        c1 = half_touch.get((h, 1), c_last)
        close_half[c0].append((h, 0))
        close_half[c1].append((h, 1))
        stop_at[min(c0, c1)].append(h)
    return perm, r_all, by_chunk, (close_half, stop_at)


def _build_bass(by_chunk, closes, total_w):
    close_half, stop_at = closes
    nc = bacc.Bacc("TRN2", target_bir_lowering=False, debug=False, num_devices=1)
    xl = nc.dram_tensor("xl", [CHUNK, XCOLS], BF16, kind="ExternalInput").ap()
    rw = nc.dram_tensor("rw", [CHUNK, total_w], FP8, kind="ExternalInput").ap()
    out_dt = BF16 if OUT_BF16 else F32
    outT = nc.dram_tensor("outT", [D_F, BSH], out_dt, kind="ExternalOutput").ap()

    with tile.TileContext(nc) as tc, ExitStack() as ctx:
        wpool = ctx.enter_context(tc.tile_pool(name="w", bufs=1))
        xpool = ctx.enter_context(tc.tile_pool(name="x", bufs=XBUFS))
        opool = ctx.enter_context(tc.tile_pool(name="o", bufs=4))
        ppool = ctx.enter_context(tc.tile_pool(name="ps", bufs=1, space="PSUM"))

        # Weights go out on the Activation DGE queue so their descriptor
        # prep overlaps the first x tile's prep on the SP queue.
        wt = wpool.tile([CHUNK, total_w], FP8, name="wt")
        nc.scalar.dma_start(wt[:], rw[:])

        psums = [ppool.tile([128, BSH], F32, name=f"psum{h}", tag=f"psum{h}")
                 for h in range(8)]

        # Zero all 8 banks: matmul with the zero weight block (start=True).
        for h in range(8):
            nc.tensor.matmul(
                psums[h][:, :],
                lhsT=wt[:, 0:CHUNK],
                rhs=wt[:, 0:BSH],
                start=True, stop=False,
            )

        c0 = 0
        for slots in SLOT_PLAN:
            xt = xpool.tile([128, slots * BSH], BF16, name="xt")
            nc.sync.dma_start(xt[:], xl[:, c0 * BSH:(c0 + slots) * BSH])
            for s in range(slots):
                c = c0 + s
                rhs = xt[:, s * BSH:(s + 1) * BSH]
                for (h, p0, m, off) in by_chunk.get(c, []):
                    nc.tensor.matmul(
                        psums[h][p0:p0 + m, :],
                        lhsT=wt[:, off:off + m],
                        rhs=rhs,
                        start=False, stop=False,
                        skip_group_check=True,
                    )
                # 1-col close: stop=True is sim-only bookkeeping and the
                # zero weights add nothing; the narrow shape keeps it off
                # the critical tail path.
                for h in stop_at.get(c, []):
                    nc.tensor.matmul(
                        psums[h][:, 0:1],
                        lhsT=wt[:, 0:CHUNK],
                        rhs=wt[:, 0:1],
                        start=False, stop=True,
                    )
                # Drain each 64-partition half-bank as soon as the sorted
                # stream passes its feature range: the copy + out-DMA (both
                # on the Activation queue, so issue order needs no
                # cross-engine hop) overlap with later chunks' matmuls.
                for (h, k) in close_half.get(c, []):
                    ot = opool.tile([64, BSH], out_dt, name="ot")
                    nc.scalar.copy(ot[:], psums[h][64 * k:64 * (k + 1), :])
                    nc.scalar.dma_start(
                        outT[128 * h + 64 * k:128 * h + 64 * (k + 1), :], ot[:])
            c0 += slots

    nc.compile()
    return nc


_CACHE = {}
_LAST_RESULTS = None


def _get_compiled(i_hash, s_hash):
    key = (i_hash.tobytes(), s_hash.tobytes())
    if key not in _CACHE:
        perm, r_all, by_chunk, closes = _build_metadata(i_hash, s_hash)
        nc = _build_bass(by_chunk, closes, r_all.shape[1])
        _CACHE[key] = (nc, perm, r_all)
    return _CACHE[key]


def predicted_ns():
    """Cost-model (TimelineSim) predicted single-core execution time in ns."""
    if not _CACHE:
        return None
    nc = next(iter(_CACHE.values()))[0]
    from concourse.timeline_sim import TimelineSim
    return int(TimelineSim(nc).simulate())


def kernel(x, s_hash, i_hash):
    x = np.asarray(x)
    in_dtype = x.dtype
    x = np.ascontiguousarray(x, dtype=np.float32)
    i_hash = np.asarray(i_hash).astype(np.int64).ravel()
    s_hash = np.asarray(s_hash).astype(np.float32).ravel()

    nc, perm, r_all = _get_compiled(i_hash, s_hash)

    # bf16 cast + bucket-sorted column permute + flat SBUF layout, all on
    # host: arr[core, p, c*512 + b] = x[core*512 + b, perm[c*128 + p]]
    xb = x.astype(ml_dtypes.bfloat16)
    xp = xb[:, perm]                                    # [4096, 16384]
    arr = xp.reshape(NCORES, BSH, N_CHUNKS, CHUNK).transpose(0, 3, 2, 1)
    arr = np.ascontiguousarray(arr)                     # [8, 128, 128, 512]
    arr = arr.reshape(NCORES, CHUNK, XCOLS)

    in_maps = [{"xl": arr[k], "rw": r_all} for k in range(NCORES)]
    res = bass_utils.run_bass_kernel_spmd(nc, in_maps, core_ids=list(range(NCORES)))
    global _LAST_RESULTS
    _LAST_RESULTS = res
    out = np.concatenate(
        [np.ascontiguousarray(res.results[k]["outT"].astype(np.float32).T)
         for k in range(NCORES)],
        axis=0,
    )
    return out.astype(in_dtype, copy=False)


# revision 20
# speedup vs baseline: 1.1986x; 1.1986x over previous
"""CountSketch kernel for Trainium2 (8 NeuronCores, SPMD data-parallel).

out[b, i_hash[j]] += x[b, j] * s_hash[j]
  x: [4096, 16384] f32, s_hash: [16384] f32, i_hash: [16384] int64 -> out [4096, 1024] f32

Strategy (batch-sharded, host-sorted bf16 layout, sequential DMA):
  - shard x by batch across 8 cores (512 rows each).
  - host computes (from the tiny i_hash/s_hash vectors) a bucket-sorted
    column order `perm`; x columns are permuted to that order, cast to
    bf16, and laid out host-side as [128 partitions, 65536] so the chunk
    for sorted position c*128+p, batch b sits at [p, c*512+b]: every
    device DMA tile is a contiguous per-partition-line slice (no gather).
  - banded +/-1 weight blocks (signs folded in) map each sorted 128-row
    chunk into its PSUM bank partitions; blocks are bf16 and only as wide
    as the PE column-tile constraints allow (base in {0,32,64}, width
    {32,64,128}).
  - each core accumulates out^T = [1024 f, 512 b] across the 128 chunks
    directly in PSUM (8 banks x [128, 512] = exactly all of PSUM); banks
    are closed, copied (cast to bf16) and DMA'd out as soon as the sorted
    stream passes their feature range, overlapping with later matmuls.
  - x tiles taper at the end (8,8,...,4,2,1,1 chunks) so the post-DMA
    matmul+drain tail is short.
  - host transposes/concatenates the 8 outT shards into [4096, 1024] f32.
"""
import numpy as np
import ml_dtypes
from contextlib import ExitStack

import concourse.bacc as bacc
import concourse.tile as tile
from concourse import mybir
from concourse import bass_utils

D_IN = 16384
D_F = 1024
B = 4096
NCORES = 8
BSH = B // NCORES          # 512 batch rows per core
CHUNK = 128                # sorted rows per matmul chunk
N_CHUNKS = D_IN // CHUNK   # 128
XCOLS = (D_IN // CHUNK) * BSH  # 65536 cols per partition of the x layout

# chunks per DMA tile: big steady-state tiles, tapered tail
SLOT_PLAN = [8] * 15 + [4, 2, 1, 1]
assert sum(SLOT_PLAN) == N_CHUNKS

F32 = mybir.dt.float32
BF16 = mybir.dt.bfloat16
FP8 = mybir.dt.float8e4   # weights dtype: signs +/-1 are exact in e4m3;
W_NP_DT = ml_dtypes.float8_e4m3  # HW-verified correct as lhsT vs bf16 rhs

MODE = "partial"           # narrow col-tiled weight blocks ("full" = [128,128])
OUT_BF16 = True            # write outT in bf16 (halves output DMA)
XBUFS = 7                  # in-flight x tiles (covers PE p-state ramp lag)


def _windows_for(fl_min, fl_max):
    """Minimal legal (p0, M) PE column windows covering [fl_min, fl_max].

    Legal combos: (0,32) (32,32) (64,32) (0,64) (64,64) (0,128).
    Returns disjoint windows covering the range.
    """
    singles = [(0, 32), (32, 32), (64, 32), (0, 64), (64, 64), (0, 128)]
    for p0, m in singles:
        if p0 <= fl_min and fl_max < p0 + m:
            return [(p0, m)]
    quads = sorted(set(range(fl_min // 32, fl_max // 32 + 1)))
    wins = []
    for q in quads:
        if q == 3:
            if (64, 32) in wins:
                wins.remove((64, 32))
            if (64, 64) not in wins:
                wins.append((64, 64))
        else:
            covered = any(p0 <= q * 32 and (q + 1) * 32 <= p0 + m for p0, m in wins)
            if not covered:
                wins.append((q * 32, 32))
    return wins


def _build_metadata(i_hash: np.ndarray, s_hash: np.ndarray):
    """Sort columns by bucket; build per-chunk banded weight blocks.

    Returns (perm, r_all, by_chunk, close_after): by_chunk[c] lists
    (bank, p0, M, off) matmul descriptors; r_all is the packed [128, total]
    bf16 weight matrix (col 0..127 = zero block); close_after[c] lists
    banks whose final touch is chunk c.
    """
    i_hash = np.asarray(i_hash).astype(np.int64).ravel()
    s_hash = np.asarray(s_hash).astype(np.float32).ravel()
    perm = np.argsort(i_hash, kind="stable")
    f_sorted = i_hash[perm]
    s_sorted = s_hash[perm]

    blocks = [np.zeros((CHUNK, CHUNK), np.float32)]  # zero block @ col 0
    off = CHUNK
    by_chunk = {}
    last_touch = {}       # bank -> last chunk touching it
    for c in range(N_CHUNKS):
        fs = f_sorted[c * CHUNK:(c + 1) * CHUNK]
        ss = s_sorted[c * CHUNK:(c + 1) * CHUNK]
        descs = []
        for h in np.unique(fs // CHUNK):
            sel = (fs // CHUNK) == h
            fl = (fs[sel] - h * CHUNK).astype(np.int64)  # local f in [0,128)
            rows = np.nonzero(sel)[0]
            last_touch[int(h)] = c
            if MODE == "full":
                wins = [(0, CHUNK)]
            else:
                wins = _windows_for(int(fl.min()), int(fl.max()))
            for (p0, m) in wins:
                wsel = (fl >= p0) & (fl < p0 + m)
                if not np.any(wsel):
                    continue
                R = np.zeros((CHUNK, m), np.float32)
                R[rows[wsel], fl[wsel] - p0] = ss[sel][wsel]
                blocks.append(R)
                descs.append((int(h), p0, m, off))
                off += m
        by_chunk[c] = descs
    r_all = np.concatenate(blocks, axis=1).astype(W_NP_DT)
    # Drain schedule: bank h drains (1-col stop matmul + full-bank copy +
    # out-DMA) right after its last touching chunk. Later chunks never write
    # that bank again (features ascend), so the drain introduces no
    # PE-stalling hazards and overlaps with subsequent matmuls.
    close_after = {c: [] for c in range(N_CHUNKS)}
    for h, c_last in last_touch.items():
        close_after[c_last].append(h)
    return perm, r_all, by_chunk, close_after


def _build_bass(by_chunk, close_after, total_w):
    nc = bacc.Bacc("TRN2", target_bir_lowering=False, debug=False, num_devices=1)
    xl = nc.dram_tensor("xl", [CHUNK, XCOLS], BF16, kind="ExternalInput").ap()
    rw = nc.dram_tensor("rw", [CHUNK, total_w], FP8, kind="ExternalInput").ap()
    out_dt = BF16 if OUT_BF16 else F32
    outT = nc.dram_tensor("outT", [D_F, BSH], out_dt, kind="ExternalOutput").ap()

    with tile.TileContext(nc) as tc, ExitStack() as ctx:
        wpool = ctx.enter_context(tc.tile_pool(name="w", bufs=1))
        xpool = ctx.enter_context(tc.tile_pool(name="x", bufs=XBUFS))
        opool = ctx.enter_context(tc.tile_pool(name="o", bufs=4))
        ppool = ctx.enter_context(tc.tile_pool(name="ps", bufs=1, space="PSUM"))

        # Weights go out on the Activation DGE queue so their descriptor
        # prep overlaps the first x tile's prep on the SP queue.
        wt = wpool.tile([CHUNK, total_w], FP8, name="wt")
        nc.scalar.dma_start(wt[:], rw[:])

        psums = [ppool.tile([128, BSH], F32, name=f"psum{h}", tag=f"psum{h}")
                 for h in range(8)]

        # Zero all 8 banks: matmul with the zero weight block (start=True).
        for h in range(8):
            nc.tensor.matmul(
                psums[h][:, :],
                lhsT=wt[:, 0:CHUNK],
                rhs=wt[:, 0:BSH],
                start=True, stop=False,
            )

        c0 = 0
        for slots in SLOT_PLAN:
            xt = xpool.tile([128, slots * BSH], BF16, name="xt")
            nc.sync.dma_start(xt[:], xl[:, c0 * BSH:(c0 + slots) * BSH])
            for s in range(slots):
                c = c0 + s
                rhs = xt[:, s * BSH:(s + 1) * BSH]
                for (h, p0, m, off) in by_chunk.get(c, []):
                    nc.tensor.matmul(
                        psums[h][p0:p0 + m, :],
                        lhsT=wt[:, off:off + m],
                        rhs=rhs,
                        start=False, stop=False,
                        skip_group_check=True,
                    )
                # Drain any bank whose feature range is complete: 1-col
                # close (stop=True is sim-only bookkeeping; the narrow shape
                # keeps it off the critical tail path), then copy + out-DMA
                # on the Activation queue (same-queue issue, no cross-engine
                # hop), overlapping with later chunks' matmuls.
                for h in close_after.get(c, []):
                    nc.tensor.matmul(
                        psums[h][:, 0:1],
                        lhsT=wt[:, 0:CHUNK],
                        rhs=wt[:, 0:1],
                        start=False, stop=True,
                    )
                    ot = opool.tile([128, BSH], out_dt, name="ot")
                    nc.scalar.copy(ot[:], psums[h][:])
                    nc.scalar.dma_start(outT[128 * h:128 * (h + 1), :], ot[:])
            c0 += slots

    nc.compile()
    return nc


_CACHE = {}
_LAST_RESULTS = None


def _get_compiled(i_hash, s_hash):
    key = (i_hash.tobytes(), s_hash.tobytes())
    if key not in _CACHE:
        perm, r_all, by_chunk, close_after = _build_metadata(i_hash, s_hash)
        nc = _build_bass(by_chunk, close_after, r_all.shape[1])
        _CACHE[key] = (nc, perm, r_all)
    return _CACHE[key]


def predicted_ns():
    """Cost-model (TimelineSim) predicted single-core execution time in ns."""
    if not _CACHE:
        return None
    nc = next(iter(_CACHE.values()))[0]
    from concourse.timeline_sim import TimelineSim
    return int(TimelineSim(nc).simulate())


def kernel(x, s_hash, i_hash):
    x = np.asarray(x)
    in_dtype = x.dtype
    x = np.ascontiguousarray(x, dtype=np.float32)
    i_hash = np.asarray(i_hash).astype(np.int64).ravel()
    s_hash = np.asarray(s_hash).astype(np.float32).ravel()

    nc, perm, r_all = _get_compiled(i_hash, s_hash)

    # bf16 cast + bucket-sorted column permute + flat SBUF layout, all on
    # host: arr[core, p, c*512 + b] = x[core*512 + b, perm[c*128 + p]]
    xb = x.astype(ml_dtypes.bfloat16)
    xp = xb[:, perm]                                    # [4096, 16384]
    arr = xp.reshape(NCORES, BSH, N_CHUNKS, CHUNK).transpose(0, 3, 2, 1)
    arr = np.ascontiguousarray(arr)                     # [8, 128, 128, 512]
    arr = arr.reshape(NCORES, CHUNK, XCOLS)

    in_maps = [{"xl": arr[k], "rw": r_all} for k in range(NCORES)]
    res = bass_utils.run_bass_kernel_spmd(nc, in_maps, core_ids=list(range(NCORES)))
    global _LAST_RESULTS
    _LAST_RESULTS = res
    out = np.concatenate(
        [np.ascontiguousarray(res.results[k]["outT"].astype(np.float32).T)
         for k in range(NCORES)],
        axis=0,
    )
    return out.astype(in_dtype, copy=False)
